# revision 29
# baseline (speedup 1.0000x reference)
"""Trainium2 Bass kernel for MQA sparse attention (nn_Attention_83356725281353).

Batch-parallel across 8 NeuronCores (4 batches each), no collectives. All
streamed tensors host-cast to bf16 (halves HBM traffic; ~4e-3 accuracy cost).
Host staging is pure layout: KV-cache roll, k transposed to [e, KV], bias
pre-permuted to the p^T tile order, v permuted per-partition-contiguous with
a ones column appended, 1/sqrt(d) folded into wq.

  per core:
    q/k_new/v_new projections (bf16 matmuls, wq streamed)
    per batch-pair, streaming kv in 2048-wide chunks:
      p^T = kT-tile.T @ qT      (kT stationary -> p lands kv-major, so the
                                 exp output feeds the o-matmul directly)
      e = exp(p^T + bias)       (DVE add + ACT exp, f32 in, bf16 out)
      o += e.T @ [v | 1]        (PSUM accumulate; ones column gives softmax
                                 denominators for free; pair via PE col-groups)
    o /= denominators; out = oT.T @ wo + bo  (wo fully SBUF-preloaded during
    attention on the ACT HWDGE ring)

Self-contained: hardcodes all shapes; builds/compiles once per process and
runs via run_bass_kernel_spmd on cores 0-7. Measured 147.7 us on hardware.
"""

import numpy as np

B, Q, DIM, H, HD, KV = 32, 4, 2048, 16, 128, 8192
NCORES = 8
BPC = B // NCORES            # 4 batches per core
BQ = BPC * Q                 # 16 (b,q) rows per core
ROWS = H * Q                 # 64 attention rows per batch
NPAIR = BPC // 2             # 2 batch-pairs per core
DT = 16                      # dim tiles (DIM/128)
KCH = 2048                   # kv chunk width
NCH = KV // KCH              # 4 chunks per batch

_CACHE = {}


def _build():
    import concourse.bass as bass
    import concourse.tile as tile
    from concourse import bacc, mybir, masks

    f32 = mybir.dt.float32
    f32r = mybir.dt.float32r
    bf16 = mybir.dt.bfloat16

    nc = bacc.Bacc("TRN2", target_bir_lowering=False, debug=False,
                   num_devices=NCORES)

    # All streamed tensors are bf16 (host-cast): halves HBM traffic, and
    # bf16 matmuls run at full PE rate.
    bf16 = mybir.dt.bfloat16
    xT = nc.dram_tensor("xT", [DIM, BQ], bf16, kind="ExternalInput").ap()
    wq = nc.dram_tensor("wq", [DIM, H * HD], bf16, kind="ExternalInput").ap()
    bq = nc.dram_tensor("bq", [1, H * HD], bf16, kind="ExternalInput").ap()
    wk = nc.dram_tensor("wk", [DIM, HD], bf16, kind="ExternalInput").ap()
    bk = nc.dram_tensor("bk", [1, HD], bf16, kind="ExternalInput").ap()
    wv = nc.dram_tensor("wv", [DIM, HD], bf16, kind="ExternalInput").ap()
    bv = nc.dram_tensor("bv", [1, HD], bf16, kind="ExternalInput").ap()
    kT = nc.dram_tensor("kT", [BPC, HD, KV], bf16, kind="ExternalInput").ap()
    vv = nc.dram_tensor("vv", [BPC, 128, KV // 128, HD + 1], bf16,
                        kind="ExternalInput").ap()
    # bias[j, p, c, (n t r)]: kv = c*2048 + n*512 + t*128 + p, r = pair-row
    bias = nc.dram_tensor("bias", [NPAIR, 128, NCH, KCH], bf16,
                          kind="ExternalInput").ap()
    wo = nc.dram_tensor("wo", [H * HD, DIM], bf16, kind="ExternalInput").ap()
    bo = nc.dram_tensor("bo", [1, DIM], bf16, kind="ExternalInput").ap()
    ones = nc.dram_tensor("ones", [1, BQ], bf16, kind="ExternalInput").ap()
    out = nc.dram_tensor("out", [BQ, DIM], f32, kind="ExternalOutput").ap()

    with tile.TileContext(nc) as tc:
        _body(tc, nc, bass, mybir, masks, xT, wq, bq, wk, bk, wv, bv, kT, vv,
              bias, wo, bo, ones, out)

    nc.compile()
    return nc


def _body(tc, nc, bass, mybir, masks, xT, wq, bq, wk, bk, wv, bv, kT, vv,
          bias, wo, bo, ones, out):
    from contextlib import ExitStack

    f32 = mybir.dt.float32
    f32r = mybir.dt.float32r
    bf16 = mybir.dt.bfloat16
    EXP = mybir.ActivationFunctionType.Exp

    with ExitStack() as octx:
        const = octx.enter_context(tc.tile_pool(name="const", bufs=1))
        wpool = octx.enter_context(tc.tile_pool(name="w", bufs=4))
        kpool = octx.enter_context(tc.tile_pool(name="kt", bufs=6))
        vpool = octx.enter_context(tc.tile_pool(name="vt", bufs=6))
        bpool = octx.enter_context(tc.tile_pool(name="bias", bufs=6))
        apool = octx.enter_context(tc.tile_pool(name="a", bufs=6))
        wopool = octx.enter_context(tc.tile_pool(name="wo", bufs=H))

        ident_f = const.tile([128, 128], f32, tag="idf")
        ident_b = const.tile([128, 128], bf16, tag="idb")
        masks.make_identity(nc, ident_f[:])
        masks.make_identity(nc, ident_b[:])
        ones16 = const.tile([1, BQ], bf16, tag="ones16")
        nc.sync.dma_start(ones16[:], ones)

        xT_sb = const.tile([128, DT * BQ], bf16, tag="xT")
        nc.sync.dma_start(xT_sb[:].rearrange("p (t m) -> p t m", t=DT),
                          xT.rearrange("(t p) m -> p t m", p=128))
        wk_sb = const.tile([128, DT * HD], bf16, tag="wk")
        nc.sync.dma_start(wk_sb[:].rearrange("p (t e) -> p t e", t=DT),
                          wk.rearrange("(t p) e -> p t e", p=128))
        wv_sb = const.tile([128, DT * HD], bf16, tag="wv")
        nc.sync.dma_start(wv_sb[:].rearrange("p (t e) -> p t e", t=DT),
                          wv.rearrange("(t p) e -> p t e", p=128))
        bq_sb = const.tile([1, H * HD], bf16, tag="bq")
        nc.sync.dma_start(bq_sb[:], bq)
        bk_sb = const.tile([1, HD], bf16, tag="bk")
        nc.sync.dma_start(bk_sb[:], bk)
        bv_sb = const.tile([1, HD], bf16, tag="bv")
        nc.sync.dma_start(bv_sb[:], bv)
        bo_sb = const.tile([1, DIM], bf16, tag="bo")
        nc.sync.dma_start(bo_sb[:], bo)

        q_sb = const.tile([BQ, H * HD], bf16, tag="q")
        kn_sb = const.tile([BQ, HD], bf16, tag="kn")
        vn_sb = const.tile([BQ, HD], bf16, tag="vn")
        # qT layout: [e, (b, h, q)] col = b*64 + h*4 + q (p-matmul moving)
        qT_sb = const.tile([128, BPC * ROWS], bf16, tag="qT")
        knT_sb = const.tile([128, BQ], bf16, tag="knT")
        # oT layout: [e=128, (h,b,q)] col = h*16 + b*4 + q
        oT_sb = const.tile([128, BPC * ROWS], bf16, tag="oT")

        # ---------------- Phase P: projections -----------------------------
        with (tc.tile_pool(name="qps", bufs=1, space="PSUM") as qps,
              tc.tile_pool(name="ptr", bufs=1, space="PSUM") as ptr):
            q_ps = qps.tile([BQ, H * HD], f32, tag="qacc")
            kv_ps = qps.tile([BQ, 2 * HD], f32, tag="kvacc")
            for t in range(DT):
                w_t = wpool.tile([128, H * HD], bf16, tag="wtile")
                nc.sync.dma_start(w_t[:], wq[t * 128:(t + 1) * 128, :])
                lhs = xT_sb[:, t * BQ:(t + 1) * BQ]
                for n in range(4):
                    nc.tensor.matmul(q_ps[:, n * 512:(n + 1) * 512], lhs,
                                     w_t[:, n * 512:(n + 1) * 512],
                                     start=(t == 0), stop=False)
                nc.tensor.matmul(kv_ps[:, 0:HD], lhs,
                                 wk_sb[:, t * HD:(t + 1) * HD],
                                 start=(t == 0), stop=False)
                nc.tensor.matmul(kv_ps[:, HD:2 * HD], lhs,
                                 wv_sb[:, t * HD:(t + 1) * HD],
                                 start=(t == 0), stop=False)
            # bias rows via ones-row matmul (K=1)
            ones_r = ones16[0:1, :]
            for n in range(4):
                nc.tensor.matmul(q_ps[:, n * 512:(n + 1) * 512], ones_r,
                                 bq_sb[0:1, n * 512:(n + 1) * 512],
                                 start=False, stop=True)
            nc.tensor.matmul(kv_ps[:, 0:HD], ones_r, bk_sb[0:1, :],
                             start=False, stop=True)
            nc.tensor.matmul(kv_ps[:, HD:2 * HD], ones_r,
                             bv_sb[0:1, :], start=False, stop=True)

            nc.vector.tensor_copy(q_sb[:], q_ps[:])
            nc.vector.tensor_copy(kn_sb[:], kv_ps[:, 0:HD])
            nc.vector.tensor_copy(vn_sb[:], kv_ps[:, HD:2 * HD])

            # transpose q: per head [16,128] -> [128,16] into one PSUM
            # tile laid out (h,b,q); then one strided copy per batch into
            # the padded qT blocks.
            qtr = ptr.tile([128, H * BQ], bf16, tag="qtr")
            for h in range(H):
                nc.tensor.transpose(qtr[:, h * BQ:(h + 1) * BQ],
                                    q_sb[:, h * HD:(h + 1) * HD],
                                    ident_b[0:BQ, 0:BQ])
            qtr_hbq = qtr[:].rearrange("p (h b q) -> p h b q", h=H, b=BPC)
            for b in range(BPC):
                dst = qT_sb[:, b * ROWS:(b + 1) * ROWS].rearrange(
                    "p (h q) -> p h q", h=H)
                nc.vector.tensor_copy(dst, qtr_hbq[:, :, b, :])
            trk = ptr.tile([128, BQ], bf16, tag="tr")
            nc.tensor.transpose(trk[:], kn_sb[:], ident_b[0:BQ, 0:BQ])
            nc.vector.tensor_copy(knT_sb[:], trk[:])

        # ---------------- Phase A: attention, per batch-pair ---------------
        # p^T layout: kT tiles are the stationary, so exp output feeds the
        # o-matmul directly (no a-transposes). Softmax denominators come from
        # the ones column appended to v on the host (o_ps col 128).
        VW = HD + 1
        with (tc.tile_pool(name="pps", bufs=4, space="PSUM") as pps,
              tc.tile_pool(name="tps", bufs=2, space="PSUM") as tps,
              tc.tile_pool(name="ops", bufs=2, space="PSUM") as ops):
            wo_tiles = []
            for j in range(NPAIR):
                b0, b1 = 2 * j, 2 * j + 1
                o_ps = ops.tile([128, VW], f32, tag="o")
                for c in range(NCH):
                    it = j * NCH + c
                    if it < H // 2:
                        for hh in range(2):
                            w_t = wopool.tile([128, DIM], bf16, tag="wot")
                            # ACT's HWDGE ring: keep the sync ring free for
                            # the latency-critical kt/v/bias stream
                            nc.scalar.dma_start(
                                w_t[:],
                                wo[(2 * it + hh) * HD:(2 * it + hh + 1) * HD, :])
                            wo_tiles.append(w_t)
                    kt0 = kpool.tile([128, KCH], bf16, tag="kt")
                    nc.sync.dma_start(kt0[:], kT[b0][:, c * KCH:(c + 1) * KCH])
                    kt1 = kpool.tile([128, KCH], bf16, tag="kt")
                    nc.sync.dma_start(kt1[:], kT[b1][:, c * KCH:(c + 1) * KCH])
                    v0 = vpool.tile([128, 16 * VW], bf16, tag="vt")
                    nc.sync.dma_start(
                        v0[:].rearrange("p (n e) -> p n e", n=16),
                        vv[b0][:, c * 16:(c + 1) * 16, :])
                    v1 = vpool.tile([128, 16 * VW], bf16, tag="vt")
                    nc.sync.dma_start(
                        v1[:].rearrange("p (n e) -> p n e", n=16),
                        vv[b1][:, c * 16:(c + 1) * 16, :])
                    bias_sb = bpool.tile([128, KCH], bf16, tag="bias")
                    # SWDGE ring: third issue path, keeps sync ring for kt/v
                    nc.gpsimd.dma_start(bias_sb[:], bias[j][:, c, :])
                    if c == NCH - 1:
                        nc.vector.tensor_copy(kt0[:, KCH - 4:KCH],
                                              knT_sb[:, b0 * 4:b0 * 4 + 4])
                        nc.vector.tensor_copy(kt1[:, KCH - 4:KCH],
                                              knT_sb[:, b1 * 4:b1 * 4 + 4])
                        nc.gpsimd.dma_start(
                            v0[124:128, 15 * VW:15 * VW + HD],
                            vn_sb[b0 * 4:b0 * 4 + 4, :])
                        nc.gpsimd.dma_start(
                            v1[124:128, 15 * VW:15 * VW + HD],
                            vn_sb[b1 * 4:b1 * 4 + 4, :])
                    for n in range(4):
                        p_ps = pps.tile([128, 512], f32, tag="p")
                        for t in range(4):
                            ko = (n * 4 + t) * 128
                            nc.tensor.matmul(
                                p_ps[:, t * 128:t * 128 + ROWS],
                                kt0[:, ko:ko + 128],
                                qT_sb[:, b0 * ROWS:(b0 + 1) * ROWS],
                                start=True, stop=True)
                            nc.tensor.matmul(
                                p_ps[:, t * 128 + ROWS:(t + 1) * 128],
                                kt1[:, ko:ko + 128],
                                qT_sb[:, b1 * ROWS:(b1 + 1) * ROWS],
                                start=True, stop=True)
                        e_sb = apool.tile([128, 512], f32, tag="e")
                        nc.vector.tensor_tensor(
                            e_sb[:], p_ps[:], bias_sb[:, n * 512:(n + 1) * 512],
                            op=mybir.AluOpType.add)
                        a_bf = apool.tile([128, 512], bf16, tag="abf")
                        nc.scalar.activation(a_bf[:], e_sb[:], EXP)
                        for t in range(4):
                            kvt = c * 16 + n * 4 + t
                            first, last = (kvt == 0), (kvt == 63)
                            vo = (n * 4 + t) * VW
                            nc.tensor.matmul(
                                o_ps[0:ROWS, :],
                                a_bf[:, t * 128:t * 128 + ROWS],
                                v0[:, vo:vo + VW], start=first, stop=last)
                            nc.tensor.matmul(
                                o_ps[ROWS:128, :],
                                a_bf[:, t * 128 + ROWS:(t + 1) * 128],
                                v1[:, vo:vo + VW], start=first, stop=last,
                                tile_position=(0, 64))
                        if j == NPAIR - 1 and c == NCH - 1:
                            # HAM keep-warm: the last chunk's drain is
                            # DVE/ACT-paced with PE nearly idle, which lets
                            # the clock gate re-throttle to 1.2 GHz and the
                            # whole output projection then runs cold. These
                            # dummy matmuls (result unused) keep the PE
                            # activity window busy through the drain.
                            for _ in range(2):
                                d_ps = pps.tile([128, 512], f32, tag="p")
                                nc.tensor.matmul(d_ps[:, :], ident_b[:],
                                                 bias_sb[:, 0:512],
                                                 start=True, stop=True)
                _finalize_pair(tc, nc, mybir, apool, tps, j, o_ps, oT_sb,
                               ident_f)
                if j == NPAIR - 1:
                    for _ in range(3):
                        d_ps = pps.tile([128, 512], f32, tag="p")
                        nc.tensor.matmul(d_ps[:, :], ident_b[:],
                                         bias_sb[:, 0:512],
                                         start=True, stop=True)

        # ---------------- Phase O: output projection ------------------------
        with tc.tile_pool(name="outps", bufs=1, space="PSUM") as outps:
            out_ps = outps.tile([BQ, DIM], f32, tag="out")
            for h in range(H):
                w_t = wo_tiles[h]
                lhs = oT_sb[:, h * BQ:(h + 1) * BQ]
                for n in range(4):
                    nc.tensor.matmul(out_ps[:, n * 512:(n + 1) * 512], lhs,
                                     w_t[:, n * 512:(n + 1) * 512],
                                     start=(h == 0), stop=False)
            ones_r = ones16[0:1, :]
            for n in range(4):
                nc.tensor.matmul(out_ps[:, n * 512:(n + 1) * 512], ones_r,
                                 bo_sb[0:1, n * 512:(n + 1) * 512],
                                 start=False, stop=True)
            out_sb = const.tile([BQ, DIM], f32, tag="osb")
            nc.vector.tensor_copy(out_sb[:], out_ps[:])
            nc.sync.dma_start(out, out_sb[:])


def _finalize_pair(tc, nc, mybir, apool, tps, j, o_ps, oT_sb, ident_f):
    f32 = mybir.dt.float32
    recip = apool.tile([128, 1], f32, tag="recip")
    nc.vector.reciprocal(recip[:], o_ps[:, HD:HD + 1])
    o_sb = apool.tile([128, HD], f32, tag="osb")
    nc.vector.tensor_scalar_mul(o_sb[:], o_ps[:, 0:HD], recip[:])
    tr = tps.tile([128, 128], f32, tag="tr")
    nc.tensor.transpose(tr[:], o_sb[:], ident_f[:])
    oT_4d = oT_sb[:].rearrange("p (h b q) -> p h b q", h=H, b=BPC)
    for b2 in range(2):
        nc.vector.tensor_copy(
            oT_4d[:, :, 2 * j + b2, :],
            tr[:, b2 * ROWS:(b2 + 1) * ROWS].rearrange(
                "p (h q) -> p h q", h=H))


def _get_nc():
    if "nc" not in _CACHE:
        _CACHE["nc"] = _build()
    return _CACHE["nc"]


def kernel(x, attn_bias, cache_k, cache_v, wq, bq, wk, bk, wv, bv, wo, bo):
    import ml_dtypes
    from concourse.bass_utils import run_bass_kernel_spmd

    nc = _get_nc()
    scale = np.float32(1.0 / np.sqrt(HD))
    bf = ml_dtypes.bfloat16

    x = np.asarray(x, np.float32)
    xT_full = np.ascontiguousarray(x.reshape(B * Q, DIM).T).astype(bf)
    wq2 = np.ascontiguousarray(
        (np.asarray(wq, np.float32) * scale).reshape(DIM, H * HD)).astype(bf)
    bq2 = np.ascontiguousarray(
        (np.asarray(bq, np.float32) * scale).reshape(1, H * HD)).astype(bf)
    wk2 = np.asarray(wk, np.float32).astype(bf)
    bk2 = np.asarray(bk, np.float32).reshape(1, HD).astype(bf)
    wv2 = np.asarray(wv, np.float32).astype(bf)
    bv2 = np.asarray(bv, np.float32).reshape(1, HD).astype(bf)
    kTh = np.ascontiguousarray(
        np.roll(np.asarray(cache_k, np.float32), -Q, axis=1)
        .transpose(0, 2, 1)).astype(bf)
    vr0 = np.roll(np.asarray(cache_v, np.float32), -Q, axis=1)
    # [B, KV, HD] -> [B, 128, KV/128, HD+1]: per-partition-contiguous runs,
    # last column = 1.0 so the o-matmul accumulates softmax denominators
    vrh4 = vr0.reshape(B, KV // 128, 128, HD).transpose(0, 2, 1, 3)
    vrh = np.ones((B, 128, KV // 128, HD + 1), np.float32)
    vrh[..., :HD] = vrh4
    vrh = np.ascontiguousarray(vrh).astype(bf)
    # bias -> [pair, p, c, (n t r)] with kv = c*2048 + n*512 + t*128 + p
    ab = np.asarray(attn_bias, np.float32).reshape(B // 2, 2, ROWS, KV)
    abP = ab.transpose(0, 3, 1, 2).reshape(B // 2, KV, 2 * ROWS)
    biasP = np.ascontiguousarray(
        abP.reshape(B // 2, NCH, 4, 4, 128, 2 * ROWS)
        .transpose(0, 4, 1, 2, 3, 5)
        .reshape(B // 2, 128, NCH, KCH)).astype(bf)
    wo2 = np.asarray(wo, np.float32).reshape(H * HD, DIM).astype(bf)
    bo2 = np.asarray(bo, np.float32).reshape(1, DIM).astype(bf)

    in_maps = []
    for c in range(NCORES):
        in_maps.append({
            "xT": np.ascontiguousarray(xT_full[:, c * BQ:(c + 1) * BQ]),
            "wq": wq2, "bq": bq2, "wk": wk2, "bk": bk2, "wv": wv2, "bv": bv2,
            "kT": np.ascontiguousarray(kTh[c * BPC:(c + 1) * BPC]),
            "vv": np.ascontiguousarray(vrh[c * BPC:(c + 1) * BPC]),
            "bias": np.ascontiguousarray(biasP[NPAIR * c:NPAIR * (c + 1)]),
            "wo": wo2, "bo": bo2,
            "ones": np.ones((1, BQ), bf),
        })

    res = run_bass_kernel_spmd(nc, in_maps, core_ids=list(range(NCORES)))
    _CACHE["last_result"] = res
    outs = [res.results[c]["out"] for c in range(NCORES)]
    return np.concatenate(outs, axis=0).reshape(B, Q, DIM).astype(np.float32)


# revision 32
# speedup vs baseline: 1.0028x; 1.0028x over previous
"""Trainium2 Bass kernel for MQA sparse attention (nn_Attention_83356725281353).

Batch-parallel across 8 NeuronCores (4 batches each), no collectives. All
streamed tensors host-cast to bf16 (halves HBM traffic; ~4e-3 accuracy cost).
Host staging is pure layout: KV-cache roll, k transposed to [e, KV], bias
pre-permuted to the p^T tile order, v permuted per-partition-contiguous with
a ones column appended, 1/sqrt(d) folded into wq.

  per core:
    q/k_new/v_new projections (bf16 matmuls, wq streamed)
    per batch-pair, streaming kv in 2048-wide chunks:
      p^T = kT-tile.T @ qT      (kT stationary -> p lands kv-major, so the
                                 exp output feeds the o-matmul directly)
      e = exp(p^T + bias)       (DVE add + ACT exp, f32 in, bf16 out)
      o += e.T @ [v | 1]        (PSUM accumulate; ones column gives softmax
                                 denominators for free; pair via PE col-groups)
    o /= denominators; out = oT.T @ wo + bo  (wo fully SBUF-preloaded during
    attention on the ACT HWDGE ring)

Self-contained: hardcodes all shapes; builds/compiles once per process and
runs via run_bass_kernel_spmd on cores 0-7. Measured 147.7 us on hardware.
"""

import numpy as np

B, Q, DIM, H, HD, KV = 32, 4, 2048, 16, 128, 8192
NCORES = 8
BPC = B // NCORES            # 4 batches per core
BQ = BPC * Q                 # 16 (b,q) rows per core
ROWS = H * Q                 # 64 attention rows per batch
NPAIR = BPC // 2             # 2 batch-pairs per core
DT = 16                      # dim tiles (DIM/128)
KCH = 4096                   # kv chunk width
NCH = KV // KCH              # chunks per batch
TPC = KCH // 128             # kv-tiles per chunk
SPC = KCH // 512             # 512-wide sub-chunks per chunk

_CACHE = {}


def _build():
    import concourse.bass as bass
    import concourse.tile as tile
    from concourse import bacc, mybir, masks

    f32 = mybir.dt.float32
    f32r = mybir.dt.float32r
    bf16 = mybir.dt.bfloat16

    nc = bacc.Bacc("TRN2", target_bir_lowering=False, debug=False,
                   num_devices=NCORES)

    # All streamed tensors are bf16 (host-cast): halves HBM traffic, and
    # bf16 matmuls run at full PE rate.
    bf16 = mybir.dt.bfloat16
    xT = nc.dram_tensor("xT", [DIM, BQ], bf16, kind="ExternalInput").ap()
    wq = nc.dram_tensor("wq", [DIM, H * HD], bf16, kind="ExternalInput").ap()
    bq = nc.dram_tensor("bq", [1, H * HD], bf16, kind="ExternalInput").ap()
    wk = nc.dram_tensor("wk", [DIM, HD], bf16, kind="ExternalInput").ap()
    bk = nc.dram_tensor("bk", [1, HD], bf16, kind="ExternalInput").ap()
    wv = nc.dram_tensor("wv", [DIM, HD], bf16, kind="ExternalInput").ap()
    bv = nc.dram_tensor("bv", [1, HD], bf16, kind="ExternalInput").ap()
    kT = nc.dram_tensor("kT", [BPC, HD, KV], bf16, kind="ExternalInput").ap()
    vv = nc.dram_tensor("vv", [BPC, 128, KV // 128, HD + 1], bf16,
                        kind="ExternalInput").ap()
    # bias[j, p, c, (n t r)]: kv = c*2048 + n*512 + t*128 + p, r = pair-row
    bias = nc.dram_tensor("bias", [NPAIR, 128, NCH, KCH], bf16,
                          kind="ExternalInput").ap()
    wo = nc.dram_tensor("wo", [H * HD, DIM], bf16, kind="ExternalInput").ap()
    bo = nc.dram_tensor("bo", [1, DIM], bf16, kind="ExternalInput").ap()
    ones = nc.dram_tensor("ones", [1, BQ], bf16, kind="ExternalInput").ap()
    out = nc.dram_tensor("out", [BQ, DIM], f32, kind="ExternalOutput").ap()

    with tile.TileContext(nc) as tc:
        _body(tc, nc, bass, mybir, masks, xT, wq, bq, wk, bk, wv, bv, kT, vv,
              bias, wo, bo, ones, out)

    nc.compile()
    return nc


def _body(tc, nc, bass, mybir, masks, xT, wq, bq, wk, bk, wv, bv, kT, vv,
          bias, wo, bo, ones, out):
    from contextlib import ExitStack

    f32 = mybir.dt.float32
    f32r = mybir.dt.float32r
    bf16 = mybir.dt.bfloat16
    EXP = mybir.ActivationFunctionType.Exp

    with ExitStack() as octx:
        const = octx.enter_context(tc.tile_pool(name="const", bufs=1))
        wpool = octx.enter_context(tc.tile_pool(name="w", bufs=4))
        kpool = octx.enter_context(tc.tile_pool(name="kt", bufs=3))
        vpool = octx.enter_context(tc.tile_pool(name="vt", bufs=3))
        bpool = octx.enter_context(tc.tile_pool(name="bias", bufs=3))
        apool = octx.enter_context(tc.tile_pool(name="a", bufs=6))
        wopool = octx.enter_context(tc.tile_pool(name="wo", bufs=H))

        ident_f = const.tile([128, 128], f32, tag="idf")
        ident_b = const.tile([128, 128], bf16, tag="idb")
        masks.make_identity(nc, ident_f[:])
        masks.make_identity(nc, ident_b[:])
        ones16 = const.tile([1, BQ], bf16, tag="ones16")
        nc.sync.dma_start(ones16[:], ones)

        xT_sb = const.tile([128, DT * BQ], bf16, tag="xT")
        nc.sync.dma_start(xT_sb[:].rearrange("p (t m) -> p t m", t=DT),
                          xT.rearrange("(t p) m -> p t m", p=128))
        wk_sb = const.tile([128, DT * HD], bf16, tag="wk")
        nc.sync.dma_start(wk_sb[:].rearrange("p (t e) -> p t e", t=DT),
                          wk.rearrange("(t p) e -> p t e", p=128))
        wv_sb = const.tile([128, DT * HD], bf16, tag="wv")
        nc.sync.dma_start(wv_sb[:].rearrange("p (t e) -> p t e", t=DT),
                          wv.rearrange("(t p) e -> p t e", p=128))
        bq_sb = const.tile([1, H * HD], bf16, tag="bq")
        nc.sync.dma_start(bq_sb[:], bq)
        bk_sb = const.tile([1, HD], bf16, tag="bk")
        nc.sync.dma_start(bk_sb[:], bk)
        bv_sb = const.tile([1, HD], bf16, tag="bv")
        nc.sync.dma_start(bv_sb[:], bv)
        bo_sb = const.tile([1, DIM], bf16, tag="bo")
        nc.sync.dma_start(bo_sb[:], bo)

        q_sb = const.tile([BQ, H * HD], bf16, tag="q")
        kn_sb = const.tile([BQ, HD], bf16, tag="kn")
        vn_sb = const.tile([BQ, HD], bf16, tag="vn")
        # qT layout: [e, (b, h, q)] col = b*64 + h*4 + q (p-matmul moving)
        qT_sb = const.tile([128, BPC * ROWS], bf16, tag="qT")
        knT_sb = const.tile([128, BQ], bf16, tag="knT")
        # oT layout: [e=128, (h,b,q)] col = h*16 + b*4 + q
        oT_sb = const.tile([128, BPC * ROWS], bf16, tag="oT")

        # ---------------- Phase P: projections -----------------------------
        with (tc.tile_pool(name="qps", bufs=1, space="PSUM") as qps,
              tc.tile_pool(name="ptr", bufs=1, space="PSUM") as ptr):
            q_ps = qps.tile([BQ, H * HD], f32, tag="qacc")
            kv_ps = qps.tile([BQ, 2 * HD], f32, tag="kvacc")
            for t in range(DT):
                w_t = wpool.tile([128, H * HD], bf16, tag="wtile")
                nc.sync.dma_start(w_t[:], wq[t * 128:(t + 1) * 128, :])
                lhs = xT_sb[:, t * BQ:(t + 1) * BQ]
                for n in range(4):
                    nc.tensor.matmul(q_ps[:, n * 512:(n + 1) * 512], lhs,
                                     w_t[:, n * 512:(n + 1) * 512],
                                     start=(t == 0), stop=False)
                nc.tensor.matmul(kv_ps[:, 0:HD], lhs,
                                 wk_sb[:, t * HD:(t + 1) * HD],
                                 start=(t == 0), stop=False)
                nc.tensor.matmul(kv_ps[:, HD:2 * HD], lhs,
                                 wv_sb[:, t * HD:(t + 1) * HD],
                                 start=(t == 0), stop=False)
            # bias rows via ones-row matmul (K=1)
            ones_r = ones16[0:1, :]
            for n in range(4):
                nc.tensor.matmul(q_ps[:, n * 512:(n + 1) * 512], ones_r,
                                 bq_sb[0:1, n * 512:(n + 1) * 512],
                                 start=False, stop=True)
            nc.tensor.matmul(kv_ps[:, 0:HD], ones_r, bk_sb[0:1, :],
                             start=False, stop=True)
            nc.tensor.matmul(kv_ps[:, HD:2 * HD], ones_r,
                             bv_sb[0:1, :], start=False, stop=True)

            nc.vector.tensor_copy(q_sb[:], q_ps[:])
            nc.vector.tensor_copy(kn_sb[:], kv_ps[:, 0:HD])
            nc.vector.tensor_copy(vn_sb[:], kv_ps[:, HD:2 * HD])

            # transpose q: per head [16,128] -> [128,16] into one PSUM
            # tile laid out (h,b,q); then one strided copy per batch into
            # the padded qT blocks.
            qtr = ptr.tile([128, H * BQ], bf16, tag="qtr")
            for h in range(H):
                nc.tensor.transpose(qtr[:, h * BQ:(h + 1) * BQ],
                                    q_sb[:, h * HD:(h + 1) * HD],
                                    ident_b[0:BQ, 0:BQ])
            qtr_hbq = qtr[:].rearrange("p (h b q) -> p h b q", h=H, b=BPC)
            for b in range(BPC):
                dst = qT_sb[:, b * ROWS:(b + 1) * ROWS].rearrange(
                    "p (h q) -> p h q", h=H)
                nc.vector.tensor_copy(dst, qtr_hbq[:, :, b, :])
            trk = ptr.tile([128, BQ], bf16, tag="tr")
            nc.tensor.transpose(trk[:], kn_sb[:], ident_b[0:BQ, 0:BQ])
            nc.vector.tensor_copy(knT_sb[:], trk[:])

        # ---------------- Phase A: attention, per batch-pair ---------------
        # p^T layout: kT tiles are the stationary, so exp output feeds the
        # o-matmul directly (no a-transposes). Softmax denominators come from
        # the ones column appended to v on the host (o_ps col 128).
        VW = HD + 1
        with (tc.tile_pool(name="pps", bufs=4, space="PSUM") as pps,
              tc.tile_pool(name="tps", bufs=2, space="PSUM") as tps,
              tc.tile_pool(name="ops", bufs=2, space="PSUM") as ops):
            wo_tiles = []
            for j in range(NPAIR):
                b0, b1 = 2 * j, 2 * j + 1
                o_ps = ops.tile([128, VW], f32, tag="o")
                for c in range(NCH):
                    it = j * NCH + c
                    if it < 4:
                        for hh in range(4):
                            w_t = wopool.tile([128, DIM], bf16, tag="wot")
                            # ACT's HWDGE ring: keep the sync ring free for
                            # the latency-critical kt/v/bias stream
                            nc.scalar.dma_start(
                                w_t[:],
                                wo[(4 * it + hh) * HD:(4 * it + hh + 1) * HD, :])
                            wo_tiles.append(w_t)
                    kt0 = kpool.tile([128, KCH], bf16, tag="kt")
                    nc.sync.dma_start(kt0[:], kT[b0][:, c * KCH:(c + 1) * KCH])
                    kt1 = kpool.tile([128, KCH], bf16, tag="kt")
                    nc.sync.dma_start(kt1[:], kT[b1][:, c * KCH:(c + 1) * KCH])
                    v0 = vpool.tile([128, TPC * VW], bf16, tag="vt")
                    nc.sync.dma_start(
                        v0[:].rearrange("p (n e) -> p n e", n=TPC),
                        vv[b0][:, c * TPC:(c + 1) * TPC, :])
                    v1 = vpool.tile([128, TPC * VW], bf16, tag="vt")
                    nc.sync.dma_start(
                        v1[:].rearrange("p (n e) -> p n e", n=TPC),
                        vv[b1][:, c * TPC:(c + 1) * TPC, :])
                    bias_sb = bpool.tile([128, KCH], bf16, tag="bias")
                    nc.sync.dma_start(bias_sb[:], bias[j][:, c, :])
                    if c == NCH - 1:
                        nc.vector.tensor_copy(kt0[:, KCH - 4:KCH],
                                              knT_sb[:, b0 * 4:b0 * 4 + 4])
                        nc.vector.tensor_copy(kt1[:, KCH - 4:KCH],
                                              knT_sb[:, b1 * 4:b1 * 4 + 4])
                        nc.gpsimd.dma_start(
                            v0[124:128, (TPC - 1) * VW:(TPC - 1) * VW + HD],
                            vn_sb[b0 * 4:b0 * 4 + 4, :])
                        nc.gpsimd.dma_start(
                            v1[124:128, (TPC - 1) * VW:(TPC - 1) * VW + HD],
                            vn_sb[b1 * 4:b1 * 4 + 4, :])
                    for n in range(SPC):
                        p_ps = pps.tile([128, 512], f32, tag="p")
                        for t in range(4):
                            ko = (n * 4 + t) * 128
                            nc.tensor.matmul(
                                p_ps[:, t * 128:t * 128 + ROWS],
                                kt0[:, ko:ko + 128],
                                qT_sb[:, b0 * ROWS:(b0 + 1) * ROWS],
                                start=True, stop=True)
                            nc.tensor.matmul(
                                p_ps[:, t * 128 + ROWS:(t + 1) * 128],
                                kt1[:, ko:ko + 128],
                                qT_sb[:, b1 * ROWS:(b1 + 1) * ROWS],
                                start=True, stop=True)
                        e_sb = apool.tile([128, 512], f32, tag="e")
                        nc.vector.tensor_tensor(
                            e_sb[:], p_ps[:], bias_sb[:, n * 512:(n + 1) * 512],
                            op=mybir.AluOpType.add)
                        a_bf = apool.tile([128, 512], bf16, tag="abf")
                        nc.scalar.activation(a_bf[:], e_sb[:], EXP)
                        for t in range(4):
                            kvt = c * TPC + n * 4 + t
                            first, last = (kvt == 0), (kvt == 63)
                            vo = (n * 4 + t) * VW
                            nc.tensor.matmul(
                                o_ps[0:ROWS, :],
                                a_bf[:, t * 128:t * 128 + ROWS],
                                v0[:, vo:vo + VW], start=first, stop=last)
                            nc.tensor.matmul(
                                o_ps[ROWS:128, :],
                                a_bf[:, t * 128 + ROWS:(t + 1) * 128],
                                v1[:, vo:vo + VW], start=first, stop=last,
                                tile_position=(0, 64))
                        if j == NPAIR - 1 and c == NCH - 1:
                            # HAM keep-warm: the last chunk's drain is
                            # DVE/ACT-paced with PE nearly idle, which lets
                            # the clock gate re-throttle to 1.2 GHz and the
                            # whole output projection then runs cold. These
                            # dummy matmuls (result unused) keep the PE
                            # activity window busy through the drain.
                            for _ in range(2):
                                d_ps = pps.tile([128, 512], f32, tag="p")
                                nc.tensor.matmul(d_ps[:, :], ident_b[:],
                                                 bias_sb[:, 0:512],
                                                 start=True, stop=True)
                _finalize_pair(tc, nc, mybir, apool, tps, j, o_ps, oT_sb,
                               ident_f)
                if j == NPAIR - 1:
                    for _ in range(3):
                        d_ps = pps.tile([128, 512], f32, tag="p")
                        nc.tensor.matmul(d_ps[:, :], ident_b[:],
                                         bias_sb[:, 0:512],
                                         start=True, stop=True)

        # ---------------- Phase O: output projection ------------------------
        with tc.tile_pool(name="outps", bufs=1, space="PSUM") as outps:
            out_ps = outps.tile([BQ, DIM], f32, tag="out")
            for h in range(H):
                w_t = wo_tiles[h]
                lhs = oT_sb[:, h * BQ:(h + 1) * BQ]
                for n in range(4):
                    nc.tensor.matmul(out_ps[:, n * 512:(n + 1) * 512], lhs,
                                     w_t[:, n * 512:(n + 1) * 512],
                                     start=(h == 0), stop=False)
            ones_r = ones16[0:1, :]
            for n in range(4):
                nc.tensor.matmul(out_ps[:, n * 512:(n + 1) * 512], ones_r,
                                 bo_sb[0:1, n * 512:(n + 1) * 512],
                                 start=False, stop=True)
            out_sb = const.tile([BQ, DIM], f32, tag="osb")
            nc.vector.tensor_copy(out_sb[:], out_ps[:])
            nc.sync.dma_start(out, out_sb[:])


def _finalize_pair(tc, nc, mybir, apool, tps, j, o_ps, oT_sb, ident_f):
    f32 = mybir.dt.float32
    recip = apool.tile([128, 1], f32, tag="recip")
    nc.vector.reciprocal(recip[:], o_ps[:, HD:HD + 1])
    o_sb = apool.tile([128, HD], f32, tag="osb")
    nc.vector.tensor_scalar_mul(o_sb[:], o_ps[:, 0:HD], recip[:])
    tr = tps.tile([128, 128], f32, tag="tr")
    nc.tensor.transpose(tr[:], o_sb[:], ident_f[:])
    oT_4d = oT_sb[:].rearrange("p (h b q) -> p h b q", h=H, b=BPC)
    for b2 in range(2):
        nc.vector.tensor_copy(
            oT_4d[:, :, 2 * j + b2, :],
            tr[:, b2 * ROWS:(b2 + 1) * ROWS].rearrange(
                "p (h q) -> p h q", h=H))


def _get_nc():
    if "nc" not in _CACHE:
        _CACHE["nc"] = _build()
    return _CACHE["nc"]


def kernel(x, attn_bias, cache_k, cache_v, wq, bq, wk, bk, wv, bv, wo, bo):
    import ml_dtypes
    from concourse.bass_utils import run_bass_kernel_spmd

    nc = _get_nc()
    scale = np.float32(1.0 / np.sqrt(HD))
    bf = ml_dtypes.bfloat16

    x = np.asarray(x, np.float32)
    xT_full = np.ascontiguousarray(x.reshape(B * Q, DIM).T).astype(bf)
    wq2 = np.ascontiguousarray(
        (np.asarray(wq, np.float32) * scale).reshape(DIM, H * HD)).astype(bf)
    bq2 = np.ascontiguousarray(
        (np.asarray(bq, np.float32) * scale).reshape(1, H * HD)).astype(bf)
    wk2 = np.asarray(wk, np.float32).astype(bf)
    bk2 = np.asarray(bk, np.float32).reshape(1, HD).astype(bf)
    wv2 = np.asarray(wv, np.float32).astype(bf)
    bv2 = np.asarray(bv, np.float32).reshape(1, HD).astype(bf)
    kTh = np.ascontiguousarray(
        np.roll(np.asarray(cache_k, np.float32), -Q, axis=1)
        .transpose(0, 2, 1)).astype(bf)
    vr0 = np.roll(np.asarray(cache_v, np.float32), -Q, axis=1)
    # [B, KV, HD] -> [B, 128, KV/128, HD+1]: per-partition-contiguous runs,
    # last column = 1.0 so the o-matmul accumulates softmax denominators
    vrh4 = vr0.reshape(B, KV // 128, 128, HD).transpose(0, 2, 1, 3)
    vrh = np.ones((B, 128, KV // 128, HD + 1), np.float32)
    vrh[..., :HD] = vrh4
    vrh = np.ascontiguousarray(vrh).astype(bf)
    # bias -> [pair, p, c, (n t r)] with kv = c*2048 + n*512 + t*128 + p
    ab = np.asarray(attn_bias, np.float32).reshape(B // 2, 2, ROWS, KV)
    abP = ab.transpose(0, 3, 1, 2).reshape(B // 2, KV, 2 * ROWS)
    biasP = np.ascontiguousarray(
        abP.reshape(B // 2, NCH, SPC, 4, 128, 2 * ROWS)
        .transpose(0, 4, 1, 2, 3, 5)
        .reshape(B // 2, 128, NCH, KCH)).astype(bf)
    wo2 = np.asarray(wo, np.float32).reshape(H * HD, DIM).astype(bf)
    bo2 = np.asarray(bo, np.float32).reshape(1, DIM).astype(bf)

    in_maps = []
    for c in range(NCORES):
        in_maps.append({
            "xT": np.ascontiguousarray(xT_full[:, c * BQ:(c + 1) * BQ]),
            "wq": wq2, "bq": bq2, "wk": wk2, "bk": bk2, "wv": wv2, "bv": bv2,
            "kT": np.ascontiguousarray(kTh[c * BPC:(c + 1) * BPC]),
            "vv": np.ascontiguousarray(vrh[c * BPC:(c + 1) * BPC]),
            "bias": np.ascontiguousarray(biasP[NPAIR * c:NPAIR * (c + 1)]),
            "wo": wo2, "bo": bo2,
            "ones": np.ones((1, BQ), bf),
        })

    res = run_bass_kernel_spmd(nc, in_maps, core_ids=list(range(NCORES)))
    _CACHE["last_result"] = res
    outs = [res.results[c]["out"] for c in range(NCORES)]
    return np.concatenate(outs, axis=0).reshape(B, Q, DIM).astype(np.float32)


# revision 34
# speedup vs baseline: 1.1434x; 1.1402x over previous
"""Trainium2 Bass kernel for MQA sparse attention (nn_Attention_83356725281353).

Batch-parallel across 8 NeuronCores (4 batches each), no collectives. All
streamed tensors host-cast to bf16 (halves HBM traffic; ~4e-3 accuracy cost).
Host staging is pure layout: KV-cache roll, k transposed to [e, KV], bias
pre-permuted to the p^T tile order, v permuted per-partition-contiguous with
a ones column appended, 1/sqrt(d) folded into wq.

  per core:
    q/k_new/v_new projections (bf16 matmuls, wq streamed)
    per batch-pair, streaming kv in 2048-wide chunks:
      p^T = kT-tile.T @ qT      (kT stationary -> p lands kv-major, so the
                                 exp output feeds the o-matmul directly)
      e = exp(p^T + bias)       (DVE add + ACT exp, f32 in, bf16 out)
      o += e.T @ [v | 1]        (PSUM accumulate; ones column gives softmax
                                 denominators for free; pair via PE col-groups)
    o /= denominators; out = oT.T @ wo + bo  (wo fully SBUF-preloaded during
    attention on the ACT HWDGE ring)

Self-contained: hardcodes all shapes; builds/compiles once per process and
runs via run_bass_kernel_spmd on cores 0-7. Measured 147.7 us on hardware.
"""

import numpy as np

B, Q, DIM, H, HD, KV = 32, 4, 2048, 16, 128, 8192
NCORES = 8
BPC = B // NCORES            # 4 batches per core
BQ = BPC * Q                 # 16 (b,q) rows per core
ROWS = H * Q                 # 64 attention rows per batch
NPAIR = BPC // 2             # 2 batch-pairs per core
DT = 16                      # dim tiles (DIM/128)
KCH = 2048                   # kv chunk width
NCH = KV // KCH              # 4 chunks per batch

_CACHE = {}


def _build():
    import concourse.bass as bass
    import concourse.tile as tile
    from concourse import bacc, mybir, masks

    f32 = mybir.dt.float32
    f32r = mybir.dt.float32r
    bf16 = mybir.dt.bfloat16

    nc = bacc.Bacc("TRN2", target_bir_lowering=False, debug=False,
                   num_devices=NCORES)

    # All streamed tensors are bf16 (host-cast): halves HBM traffic, and
    # bf16 matmuls run at full PE rate.
    bf16 = mybir.dt.bfloat16
    xT = nc.dram_tensor("xT", [DIM, BQ], bf16, kind="ExternalInput").ap()
    wq = nc.dram_tensor("wq", [DIM, H * HD], bf16, kind="ExternalInput").ap()
    bq = nc.dram_tensor("bq", [1, H * HD], bf16, kind="ExternalInput").ap()
    wk = nc.dram_tensor("wk", [DIM, HD], bf16, kind="ExternalInput").ap()
    bk = nc.dram_tensor("bk", [1, HD], bf16, kind="ExternalInput").ap()
    wv = nc.dram_tensor("wv", [DIM, HD], bf16, kind="ExternalInput").ap()
    bv = nc.dram_tensor("bv", [1, HD], bf16, kind="ExternalInput").ap()
    kT = nc.dram_tensor("kT", [BPC, HD, KV], bf16, kind="ExternalInput").ap()
    vv = nc.dram_tensor("vv", [BPC, 128, KV // 128, HD + 1], bf16,
                        kind="ExternalInput").ap()
    # bias[j, p, c, (n t r)]: kv = c*2048 + n*512 + t*128 + p, r = pair-row
    bias = nc.dram_tensor("bias", [NPAIR, 128, NCH, KCH], bf16,
                          kind="ExternalInput").ap()
    wo = nc.dram_tensor("wo", [H * HD, DIM], bf16, kind="ExternalInput").ap()
    bo = nc.dram_tensor("bo", [1, DIM], bf16, kind="ExternalInput").ap()
    ones = nc.dram_tensor("ones", [1, BQ], bf16, kind="ExternalInput").ap()
    out = nc.dram_tensor("out", [BQ, DIM], f32, kind="ExternalOutput").ap()

    with tile.TileContext(nc) as tc:
        _body(tc, nc, bass, mybir, masks, xT, wq, bq, wk, bk, wv, bv, kT, vv,
              bias, wo, bo, ones, out)

    nc.compile()
    return nc


def _body(tc, nc, bass, mybir, masks, xT, wq, bq, wk, bk, wv, bv, kT, vv,
          bias, wo, bo, ones, out):
    from contextlib import ExitStack

    f32 = mybir.dt.float32
    f32r = mybir.dt.float32r
    bf16 = mybir.dt.bfloat16
    EXP = mybir.ActivationFunctionType.Exp

    with ExitStack() as octx:
        const = octx.enter_context(tc.tile_pool(name="const", bufs=1))
        wpool = octx.enter_context(tc.tile_pool(name="w", bufs=4))
        kpool = octx.enter_context(tc.tile_pool(name="kt", bufs=6))
        vpool = octx.enter_context(tc.tile_pool(name="vt", bufs=6))
        bpool = octx.enter_context(tc.tile_pool(name="bias", bufs=4))
        apool = octx.enter_context(tc.tile_pool(name="a", bufs=6))
        wopool = octx.enter_context(tc.tile_pool(name="wo", bufs=H))

        ident_f = const.tile([128, 128], f32, tag="idf")
        ident_b = const.tile([128, 128], bf16, tag="idb")
        masks.make_identity(nc, ident_f[:])
        masks.make_identity(nc, ident_b[:])
        ones16 = const.tile([1, BQ], bf16, tag="ones16")
        nc.sync.dma_start(ones16[:], ones)

        xT_sb = const.tile([128, DT * BQ], bf16, tag="xT")
        nc.sync.dma_start(xT_sb[:].rearrange("p (t m) -> p t m", t=DT),
                          xT.rearrange("(t p) m -> p t m", p=128))
        wk_sb = const.tile([128, DT * HD], bf16, tag="wk")
        nc.sync.dma_start(wk_sb[:].rearrange("p (t e) -> p t e", t=DT),
                          wk.rearrange("(t p) e -> p t e", p=128))
        wv_sb = const.tile([128, DT * HD], bf16, tag="wv")
        nc.sync.dma_start(wv_sb[:].rearrange("p (t e) -> p t e", t=DT),
                          wv.rearrange("(t p) e -> p t e", p=128))
        bq_sb = const.tile([1, H * HD], bf16, tag="bq")
        nc.sync.dma_start(bq_sb[:], bq)
        bk_sb = const.tile([1, HD], bf16, tag="bk")
        nc.sync.dma_start(bk_sb[:], bk)
        bv_sb = const.tile([1, HD], bf16, tag="bv")
        nc.sync.dma_start(bv_sb[:], bv)
        bo_sb = const.tile([1, DIM], bf16, tag="bo")
        nc.sync.dma_start(bo_sb[:], bo)

        q_sb = const.tile([BQ, H * HD], bf16, tag="q")
        kn_sb = const.tile([BQ, HD], bf16, tag="kn")
        vn_sb = const.tile([BQ, HD], bf16, tag="vn")
        # qT layout: [e, (b, h, q)] col = b*64 + h*4 + q (p-matmul moving)
        qT_sb = const.tile([128, BPC * ROWS], bf16, tag="qT")
        knT_sb = const.tile([128, BQ], bf16, tag="knT")
        # oT layout: [e=128, (h,b,q)] col = h*16 + b*4 + q
        oT_sb = const.tile([128, BPC * ROWS], bf16, tag="oT")

        # ---------------- Phase P: projections -----------------------------
        with (tc.tile_pool(name="qps", bufs=1, space="PSUM") as qps,
              tc.tile_pool(name="ptr", bufs=1, space="PSUM") as ptr):
            q_ps = qps.tile([BQ, H * HD], f32, tag="qacc")
            kv_ps = qps.tile([BQ, 2 * HD], f32, tag="kvacc")
            for t in range(DT):
                w_t = wpool.tile([128, H * HD], bf16, tag="wtile")
                nc.sync.dma_start(w_t[:], wq[t * 128:(t + 1) * 128, :])
                lhs = xT_sb[:, t * BQ:(t + 1) * BQ]
                for n in range(4):
                    nc.tensor.matmul(q_ps[:, n * 512:(n + 1) * 512], lhs,
                                     w_t[:, n * 512:(n + 1) * 512],
                                     start=(t == 0), stop=False)
                nc.tensor.matmul(kv_ps[:, 0:HD], lhs,
                                 wk_sb[:, t * HD:(t + 1) * HD],
                                 start=(t == 0), stop=False)
                nc.tensor.matmul(kv_ps[:, HD:2 * HD], lhs,
                                 wv_sb[:, t * HD:(t + 1) * HD],
                                 start=(t == 0), stop=False)
            # bias rows via ones-row matmul (K=1)
            ones_r = ones16[0:1, :]
            for n in range(4):
                nc.tensor.matmul(q_ps[:, n * 512:(n + 1) * 512], ones_r,
                                 bq_sb[0:1, n * 512:(n + 1) * 512],
                                 start=False, stop=True)
            nc.tensor.matmul(kv_ps[:, 0:HD], ones_r, bk_sb[0:1, :],
                             start=False, stop=True)
            nc.tensor.matmul(kv_ps[:, HD:2 * HD], ones_r,
                             bv_sb[0:1, :], start=False, stop=True)

            nc.vector.tensor_copy(q_sb[:], q_ps[:])
            nc.vector.tensor_copy(kn_sb[:], kv_ps[:, 0:HD])
            nc.vector.tensor_copy(vn_sb[:], kv_ps[:, HD:2 * HD])

            # transpose q: per head [16,128] -> [128,16] into one PSUM
            # tile laid out (h,b,q); then one strided copy per batch into
            # the padded qT blocks.
            qtr = ptr.tile([128, H * BQ], bf16, tag="qtr")
            for h in range(H):
                nc.tensor.transpose(qtr[:, h * BQ:(h + 1) * BQ],
                                    q_sb[:, h * HD:(h + 1) * HD],
                                    ident_b[0:BQ, 0:BQ])
            qtr_hbq = qtr[:].rearrange("p (h b q) -> p h b q", h=H, b=BPC)
            for b in range(BPC):
                dst = qT_sb[:, b * ROWS:(b + 1) * ROWS].rearrange(
                    "p (h q) -> p h q", h=H)
                nc.vector.tensor_copy(dst, qtr_hbq[:, :, b, :])
            trk = ptr.tile([128, BQ], bf16, tag="tr")
            nc.tensor.transpose(trk[:], kn_sb[:], ident_b[0:BQ, 0:BQ])
            nc.vector.tensor_copy(knT_sb[:], trk[:])

        # ---------------- Phase A: attention, per batch-pair ---------------
        # p^T layout: kT tiles are the stationary, so exp output feeds the
        # o-matmul directly (no a-transposes). Softmax denominators come from
        # the ones column appended to v on the host (o_ps col 128).
        VW = HD + 1
        with (tc.tile_pool(name="pps", bufs=4, space="PSUM") as pps,
              tc.tile_pool(name="tps", bufs=2, space="PSUM") as tps,
              tc.tile_pool(name="ops", bufs=2, space="PSUM") as ops):
            wo_tiles = []
            for j in range(NPAIR):
                b0, b1 = 2 * j, 2 * j + 1
                o_ps = ops.tile([128, VW], f32, tag="o")
                for c in range(NCH):
                    it = j * NCH + c
                    if it < H // 2:
                        for hh in range(2):
                            w_t = wopool.tile([128, DIM], bf16, tag="wot")
                            # ACT's HWDGE ring: keep the sync ring free for
                            # the latency-critical kt/v/bias stream
                            nc.scalar.dma_start(
                                w_t[:],
                                wo[(2 * it + hh) * HD:(2 * it + hh + 1) * HD, :])
                            wo_tiles.append(w_t)
                    kt0 = kpool.tile([128, KCH], bf16, tag="kt")
                    nc.sync.dma_start(kt0[:], kT[b0][:, c * KCH:(c + 1) * KCH])
                    kt1 = kpool.tile([128, KCH], bf16, tag="kt")
                    nc.sync.dma_start(kt1[:], kT[b1][:, c * KCH:(c + 1) * KCH])
                    v0 = vpool.tile([128, 16 * VW], bf16, tag="vt")
                    nc.sync.dma_start(
                        v0[:].rearrange("p (n e) -> p n e", n=16),
                        vv[b0][:, c * 16:(c + 1) * 16, :])
                    v1 = vpool.tile([128, 16 * VW], bf16, tag="vt")
                    nc.sync.dma_start(
                        v1[:].rearrange("p (n e) -> p n e", n=16),
                        vv[b1][:, c * 16:(c + 1) * 16, :])
                    bias_sb = bpool.tile([128, KCH], bf16, tag="bias")
                    nc.sync.dma_start(bias_sb[:], bias[j][:, c, :])
                    if c == NCH - 1:
                        nc.vector.tensor_copy(kt0[:, KCH - 4:KCH],
                                              knT_sb[:, b0 * 4:b0 * 4 + 4])
                        nc.vector.tensor_copy(kt1[:, KCH - 4:KCH],
                                              knT_sb[:, b1 * 4:b1 * 4 + 4])
                        nc.gpsimd.dma_start(
                            v0[124:128, 15 * VW:15 * VW + HD],
                            vn_sb[b0 * 4:b0 * 4 + 4, :])
                        nc.gpsimd.dma_start(
                            v1[124:128, 15 * VW:15 * VW + HD],
                            vn_sb[b1 * 4:b1 * 4 + 4, :])
                    for n in range(4):
                        p_ps = pps.tile([128, 512], f32, tag="p")
                        for t in range(4):
                            ko = (n * 4 + t) * 128
                            nc.tensor.matmul(
                                p_ps[:, t * 128:t * 128 + ROWS],
                                kt0[:, ko:ko + 128],
                                qT_sb[:, b0 * ROWS:(b0 + 1) * ROWS],
                                start=True, stop=True)
                            nc.tensor.matmul(
                                p_ps[:, t * 128 + ROWS:(t + 1) * 128],
                                kt1[:, ko:ko + 128],
                                qT_sb[:, b1 * ROWS:(b1 + 1) * ROWS],
                                start=True, stop=True)
                        # final chunk: 256-wide halves shorten the
                        # DVE->ACT-paced drain; full-width elsewhere (the
                        # global split was measured slower)
                        nhalf = 2 if (j == NPAIR - 1 and c == NCH - 1) else 1
                        W = 512 // nhalf
                        for hf in range(nhalf):
                            co = hf * W
                            e_sb = apool.tile([128, W], f32, tag="e")
                            nc.vector.tensor_tensor(
                                e_sb[:], p_ps[:, co:co + W],
                                bias_sb[:, n * 512 + co:n * 512 + co + W],
                                op=mybir.AluOpType.add)
                            a_bf = apool.tile([128, W], bf16, tag="abf")
                            nc.scalar.activation(a_bf[:], e_sb[:], EXP)
                            for t2 in range(W // 128):
                                t = hf * (W // 128) + t2
                                kvt = c * 16 + n * 4 + t
                                first, last = (kvt == 0), (kvt == 63)
                                vo = (n * 4 + t) * VW
                                nc.tensor.matmul(
                                    o_ps[0:ROWS, :],
                                    a_bf[:, t2 * 128:t2 * 128 + ROWS],
                                    v0[:, vo:vo + VW], start=first, stop=last)
                                nc.tensor.matmul(
                                    o_ps[ROWS:128, :],
                                    a_bf[:, t2 * 128 + ROWS:(t2 + 1) * 128],
                                    v1[:, vo:vo + VW], start=first, stop=last,
                                    tile_position=(0, 64))
                        if j == NPAIR - 1 and c == NCH - 1:
                            # HAM keep-warm: the last chunk's drain is
                            # DVE/ACT-paced with PE nearly idle, which lets
                            # the clock gate re-throttle to 1.2 GHz and the
                            # whole output projection then runs cold. These
                            # dummy matmuls (result unused) keep the PE
                            # activity window busy through the drain.
                            for _ in range(2):
                                d_ps = pps.tile([128, 512], f32, tag="p")
                                nc.tensor.matmul(d_ps[:, :], ident_b[:],
                                                 bias_sb[:, 0:512],
                                                 start=True, stop=True)
                _finalize_pair(tc, nc, mybir, apool, tps, j, o_ps, oT_sb,
                               ident_f)
                if j == NPAIR - 1:
                    for _ in range(3):
                        d_ps = pps.tile([128, 512], f32, tag="p")
                        nc.tensor.matmul(d_ps[:, :], ident_b[:],
                                         bias_sb[:, 0:512],
                                         start=True, stop=True)

        # ---------------- Phase O: output projection ------------------------
        with tc.tile_pool(name="outps", bufs=1, space="PSUM") as outps:
            out_ps = outps.tile([BQ, DIM], f32, tag="out")
            for h in range(H):
                w_t = wo_tiles[h]
                lhs = oT_sb[:, h * BQ:(h + 1) * BQ]
                for n in range(4):
                    nc.tensor.matmul(out_ps[:, n * 512:(n + 1) * 512], lhs,
                                     w_t[:, n * 512:(n + 1) * 512],
                                     start=(h == 0), stop=False)
            ones_r = ones16[0:1, :]
            for n in range(4):
                nc.tensor.matmul(out_ps[:, n * 512:(n + 1) * 512], ones_r,
                                 bo_sb[0:1, n * 512:(n + 1) * 512],
                                 start=False, stop=True)
            out_sb = const.tile([BQ, DIM], f32, tag="osb")
            nc.vector.tensor_copy(out_sb[:], out_ps[:])
            nc.sync.dma_start(out, out_sb[:])


def _finalize_pair(tc, nc, mybir, apool, tps, j, o_ps, oT_sb, ident_f):
    f32 = mybir.dt.float32
    recip = apool.tile([128, 1], f32, tag="recip")
    nc.vector.reciprocal(recip[:], o_ps[:, HD:HD + 1])
    o_sb = apool.tile([128, HD], f32, tag="osb")
    nc.vector.tensor_scalar_mul(o_sb[:], o_ps[:, 0:HD], recip[:])
    tr = tps.tile([128, 128], f32, tag="tr")
    nc.tensor.transpose(tr[:], o_sb[:], ident_f[:])
    oT_4d = oT_sb[:].rearrange("p (h b q) -> p h b q", h=H, b=BPC)
    for b2 in range(2):
        nc.vector.tensor_copy(
            oT_4d[:, :, 2 * j + b2, :],
            tr[:, b2 * ROWS:(b2 + 1) * ROWS].rearrange(
                "p (h q) -> p h q", h=H))


def _get_nc():
    if "nc" not in _CACHE:
        _CACHE["nc"] = _build()
    return _CACHE["nc"]


def kernel(x, attn_bias, cache_k, cache_v, wq, bq, wk, bk, wv, bv, wo, bo):
    import ml_dtypes
    from concourse.bass_utils import run_bass_kernel_spmd

    nc = _get_nc()
    scale = np.float32(1.0 / np.sqrt(HD))
    bf = ml_dtypes.bfloat16

    x = np.asarray(x, np.float32)
    xT_full = np.ascontiguousarray(x.reshape(B * Q, DIM).T).astype(bf)
    wq2 = np.ascontiguousarray(
        (np.asarray(wq, np.float32) * scale).reshape(DIM, H * HD)).astype(bf)
    bq2 = np.ascontiguousarray(
        (np.asarray(bq, np.float32) * scale).reshape(1, H * HD)).astype(bf)
    wk2 = np.asarray(wk, np.float32).astype(bf)
    bk2 = np.asarray(bk, np.float32).reshape(1, HD).astype(bf)
    wv2 = np.asarray(wv, np.float32).astype(bf)
    bv2 = np.asarray(bv, np.float32).reshape(1, HD).astype(bf)
    kTh = np.ascontiguousarray(
        np.roll(np.asarray(cache_k, np.float32), -Q, axis=1)
        .transpose(0, 2, 1)).astype(bf)
    vr0 = np.roll(np.asarray(cache_v, np.float32), -Q, axis=1)
    # [B, KV, HD] -> [B, 128, KV/128, HD+1]: per-partition-contiguous runs,
    # last column = 1.0 so the o-matmul accumulates softmax denominators
    vrh4 = vr0.reshape(B, KV // 128, 128, HD).transpose(0, 2, 1, 3)
    vrh = np.ones((B, 128, KV // 128, HD + 1), np.float32)
    vrh[..., :HD] = vrh4
    vrh = np.ascontiguousarray(vrh).astype(bf)
    # bias -> [pair, p, c, (n t r)] with kv = c*2048 + n*512 + t*128 + p
    ab = np.asarray(attn_bias, np.float32).reshape(B // 2, 2, ROWS, KV)
    abP = ab.transpose(0, 3, 1, 2).reshape(B // 2, KV, 2 * ROWS)
    biasP = np.ascontiguousarray(
        abP.reshape(B // 2, NCH, 4, 4, 128, 2 * ROWS)
        .transpose(0, 4, 1, 2, 3, 5)
        .reshape(B // 2, 128, NCH, KCH)).astype(bf)
    wo2 = np.asarray(wo, np.float32).reshape(H * HD, DIM).astype(bf)
    bo2 = np.asarray(bo, np.float32).reshape(1, DIM).astype(bf)

    in_maps = []
    for c in range(NCORES):
        in_maps.append({
            "xT": np.ascontiguousarray(xT_full[:, c * BQ:(c + 1) * BQ]),
            "wq": wq2, "bq": bq2, "wk": wk2, "bk": bk2, "wv": wv2, "bv": bv2,
            "kT": np.ascontiguousarray(kTh[c * BPC:(c + 1) * BPC]),
            "vv": np.ascontiguousarray(vrh[c * BPC:(c + 1) * BPC]),
            "bias": np.ascontiguousarray(biasP[NPAIR * c:NPAIR * (c + 1)]),
            "wo": wo2, "bo": bo2,
            "ones": np.ones((1, BQ), bf),
        })

    res = run_bass_kernel_spmd(nc, in_maps, core_ids=list(range(NCORES)))
    _CACHE["last_result"] = res
    outs = [res.results[c]["out"] for c in range(NCORES)]
    return np.concatenate(outs, axis=0).reshape(B, Q, DIM).astype(np.float32)


# revision 36
# speedup vs baseline: 1.1491x; 1.0049x over previous
"""Trainium2 Bass kernel for MQA sparse attention (nn_Attention_83356725281353).

Batch-parallel across 8 NeuronCores (4 batches each), no collectives. All
streamed tensors host-cast to bf16 (halves HBM traffic; ~4e-3 accuracy cost).
Host staging is pure layout: KV-cache roll, k transposed to [e, KV], bias
pre-permuted to the p^T tile order, v permuted per-partition-contiguous with
a ones column appended, 1/sqrt(d) folded into wq.

  per core:
    q/k_new/v_new projections (bf16 matmuls, wq streamed)
    per batch-pair, streaming kv in 2048-wide chunks:
      p^T = kT-tile.T @ qT      (kT stationary -> p lands kv-major, so the
                                 exp output feeds the o-matmul directly)
      e = exp(p^T + bias)       (DVE add + ACT exp, f32 in, bf16 out)
      o += e.T @ [v | 1]        (PSUM accumulate; ones column gives softmax
                                 denominators for free; pair via PE col-groups)
    o /= denominators; out = oT.T @ wo + bo  (wo fully SBUF-preloaded during
    attention on the ACT HWDGE ring)

Self-contained: hardcodes all shapes; builds/compiles once per process and
runs via run_bass_kernel_spmd on cores 0-7. Measured 147.7 us on hardware.
"""

import numpy as np

B, Q, DIM, H, HD, KV = 32, 4, 2048, 16, 128, 8192
NCORES = 8
BPC = B // NCORES            # 4 batches per core
BQ = BPC * Q                 # 16 (b,q) rows per core
ROWS = H * Q                 # 64 attention rows per batch
NPAIR = BPC // 2             # 2 batch-pairs per core
DT = 16                      # dim tiles (DIM/128)
KCH = 2048                   # kv chunk width
NCH = KV // KCH              # 4 chunks per batch

_CACHE = {}


def _build():
    import concourse.bass as bass
    import concourse.tile as tile
    from concourse import bacc, mybir, masks

    f32 = mybir.dt.float32
    f32r = mybir.dt.float32r
    bf16 = mybir.dt.bfloat16

    nc = bacc.Bacc("TRN2", target_bir_lowering=False, debug=False,
                   num_devices=NCORES)

    # All streamed tensors are bf16 (host-cast): halves HBM traffic, and
    # bf16 matmuls run at full PE rate.
    bf16 = mybir.dt.bfloat16
    xT = nc.dram_tensor("xT", [DIM, BQ], bf16, kind="ExternalInput").ap()
    wq = nc.dram_tensor("wq", [DIM, H * HD], bf16, kind="ExternalInput").ap()
    bq = nc.dram_tensor("bq", [1, H * HD], bf16, kind="ExternalInput").ap()
    wk = nc.dram_tensor("wk", [DIM, HD], bf16, kind="ExternalInput").ap()
    bk = nc.dram_tensor("bk", [1, HD], bf16, kind="ExternalInput").ap()
    wv = nc.dram_tensor("wv", [DIM, HD], bf16, kind="ExternalInput").ap()
    bv = nc.dram_tensor("bv", [1, HD], bf16, kind="ExternalInput").ap()
    kT = nc.dram_tensor("kT", [BPC, HD, KV], bf16, kind="ExternalInput").ap()
    vv = nc.dram_tensor("vv", [BPC, 128, KV // 128, HD + 1], bf16,
                        kind="ExternalInput").ap()
    # bias[j, p, c, (n t r)]: kv = c*2048 + n*512 + t*128 + p, r = pair-row
    bias = nc.dram_tensor("bias", [NPAIR, 128, NCH, KCH], bf16,
                          kind="ExternalInput").ap()
    wo = nc.dram_tensor("wo", [H * HD, DIM], bf16, kind="ExternalInput").ap()
    bo = nc.dram_tensor("bo", [1, DIM], bf16, kind="ExternalInput").ap()
    ones = nc.dram_tensor("ones", [1, BQ], bf16, kind="ExternalInput").ap()
    out = nc.dram_tensor("out", [BQ, DIM], f32, kind="ExternalOutput").ap()

    with tile.TileContext(nc) as tc:
        _body(tc, nc, bass, mybir, masks, xT, wq, bq, wk, bk, wv, bv, kT, vv,
              bias, wo, bo, ones, out)

    nc.compile()
    return nc


def _body(tc, nc, bass, mybir, masks, xT, wq, bq, wk, bk, wv, bv, kT, vv,
          bias, wo, bo, ones, out):
    from contextlib import ExitStack

    f32 = mybir.dt.float32
    f32r = mybir.dt.float32r
    bf16 = mybir.dt.bfloat16
    EXP = mybir.ActivationFunctionType.Exp

    with ExitStack() as octx:
        const = octx.enter_context(tc.tile_pool(name="const", bufs=1))
        wpool = octx.enter_context(tc.tile_pool(name="w", bufs=4))
        kpool = octx.enter_context(tc.tile_pool(name="kt", bufs=6))
        vpool = octx.enter_context(tc.tile_pool(name="vt", bufs=6))
        bpool = octx.enter_context(tc.tile_pool(name="bias", bufs=4))
        apool = octx.enter_context(tc.tile_pool(name="a", bufs=6))
        wopool = octx.enter_context(tc.tile_pool(name="wo", bufs=H))

        ident_f = const.tile([128, 128], f32, tag="idf")
        ident_b = const.tile([128, 128], bf16, tag="idb")
        masks.make_identity(nc, ident_f[:])
        masks.make_identity(nc, ident_b[:])
        ones16 = const.tile([1, BQ], bf16, tag="ones16")
        nc.sync.dma_start(ones16[:], ones)

        xT_sb = const.tile([128, DT * BQ], bf16, tag="xT")
        nc.sync.dma_start(xT_sb[:].rearrange("p (t m) -> p t m", t=DT),
                          xT.rearrange("(t p) m -> p t m", p=128))
        wk_sb = const.tile([128, DT * HD], bf16, tag="wk")
        nc.sync.dma_start(wk_sb[:].rearrange("p (t e) -> p t e", t=DT),
                          wk.rearrange("(t p) e -> p t e", p=128))
        wv_sb = const.tile([128, DT * HD], bf16, tag="wv")
        nc.sync.dma_start(wv_sb[:].rearrange("p (t e) -> p t e", t=DT),
                          wv.rearrange("(t p) e -> p t e", p=128))
        bq_sb = const.tile([1, H * HD], bf16, tag="bq")
        nc.sync.dma_start(bq_sb[:], bq)
        bk_sb = const.tile([1, HD], bf16, tag="bk")
        nc.sync.dma_start(bk_sb[:], bk)
        bv_sb = const.tile([1, HD], bf16, tag="bv")
        nc.sync.dma_start(bv_sb[:], bv)
        bo_sb = const.tile([1, DIM], bf16, tag="bo")
        nc.sync.dma_start(bo_sb[:], bo)

        q_sb = const.tile([BQ, H * HD], bf16, tag="q")
        kn_sb = const.tile([BQ, HD], bf16, tag="kn")
        vn_sb = const.tile([BQ, HD], bf16, tag="vn")
        # qT layout: [e, (b, h, q)] col = b*64 + h*4 + q (p-matmul moving)
        qT_sb = const.tile([128, BPC * ROWS], bf16, tag="qT")
        knT_sb = const.tile([128, BQ], bf16, tag="knT")
        # oT layout: [e=128, (h,b,q)] col = h*16 + b*4 + q
        oT_sb = const.tile([128, BPC * ROWS], bf16, tag="oT")

        # ---------------- Phase P: projections -----------------------------
        with (tc.tile_pool(name="qps", bufs=1, space="PSUM") as qps,
              tc.tile_pool(name="ptr", bufs=1, space="PSUM") as ptr):
            q_ps = qps.tile([BQ, H * HD], f32, tag="qacc")
            kv_ps = qps.tile([BQ, 2 * HD], f32, tag="kvacc")
            # HAM warm-up: PE is otherwise idle during the first wq DMA, so
            # the projections would run at the cold 1.2 GHz clock; ~30 dummy
            # matmuls ramp the clock gate before real work arrives.
            for _ in range(30):
                d_ps = ptr.tile([128, 128], f32, tag="warm")
                nc.tensor.matmul(d_ps[:, :], ident_b[:], ident_b[:],
                                 start=True, stop=True)
            for t in range(DT):
                w_t = wpool.tile([128, H * HD], bf16, tag="wtile")
                nc.sync.dma_start(w_t[:], wq[t * 128:(t + 1) * 128, :])
                lhs = xT_sb[:, t * BQ:(t + 1) * BQ]
                for n in range(4):
                    nc.tensor.matmul(q_ps[:, n * 512:(n + 1) * 512], lhs,
                                     w_t[:, n * 512:(n + 1) * 512],
                                     start=(t == 0), stop=False)
                nc.tensor.matmul(kv_ps[:, 0:HD], lhs,
                                 wk_sb[:, t * HD:(t + 1) * HD],
                                 start=(t == 0), stop=False)
                nc.tensor.matmul(kv_ps[:, HD:2 * HD], lhs,
                                 wv_sb[:, t * HD:(t + 1) * HD],
                                 start=(t == 0), stop=False)
            # bias rows via ones-row matmul (K=1)
            ones_r = ones16[0:1, :]
            for n in range(4):
                nc.tensor.matmul(q_ps[:, n * 512:(n + 1) * 512], ones_r,
                                 bq_sb[0:1, n * 512:(n + 1) * 512],
                                 start=False, stop=True)
            nc.tensor.matmul(kv_ps[:, 0:HD], ones_r, bk_sb[0:1, :],
                             start=False, stop=True)
            nc.tensor.matmul(kv_ps[:, HD:2 * HD], ones_r,
                             bv_sb[0:1, :], start=False, stop=True)

            nc.vector.tensor_copy(q_sb[:], q_ps[:])
            nc.vector.tensor_copy(kn_sb[:], kv_ps[:, 0:HD])
            nc.vector.tensor_copy(vn_sb[:], kv_ps[:, HD:2 * HD])

            # transpose q: per head [16,128] -> [128,16] into one PSUM
            # tile laid out (h,b,q); then one strided copy per batch into
            # the padded qT blocks.
            qtr = ptr.tile([128, H * BQ], bf16, tag="qtr")
            for h in range(H):
                nc.tensor.transpose(qtr[:, h * BQ:(h + 1) * BQ],
                                    q_sb[:, h * HD:(h + 1) * HD],
                                    ident_b[0:BQ, 0:BQ])
            qtr_hbq = qtr[:].rearrange("p (h b q) -> p h b q", h=H, b=BPC)
            for b in range(BPC):
                dst = qT_sb[:, b * ROWS:(b + 1) * ROWS].rearrange(
                    "p (h q) -> p h q", h=H)
                nc.vector.tensor_copy(dst, qtr_hbq[:, :, b, :])
            trk = ptr.tile([128, BQ], bf16, tag="tr")
            nc.tensor.transpose(trk[:], kn_sb[:], ident_b[0:BQ, 0:BQ])
            nc.vector.tensor_copy(knT_sb[:], trk[:])

        # ---------------- Phase A: attention, per batch-pair ---------------
        # p^T layout: kT tiles are the stationary, so exp output feeds the
        # o-matmul directly (no a-transposes). Softmax denominators come from
        # the ones column appended to v on the host (o_ps col 128).
        VW = HD + 1
        with (tc.tile_pool(name="pps", bufs=4, space="PSUM") as pps,
              tc.tile_pool(name="tps", bufs=2, space="PSUM") as tps,
              tc.tile_pool(name="ops", bufs=2, space="PSUM") as ops):
            wo_tiles = []
            for j in range(NPAIR):
                b0, b1 = 2 * j, 2 * j + 1
                o_ps = ops.tile([128, VW], f32, tag="o")
                for c in range(NCH):
                    it = j * NCH + c
                    if it < H // 2:
                        for hh in range(2):
                            w_t = wopool.tile([128, DIM], bf16, tag="wot")
                            # ACT's HWDGE ring: keep the sync ring free for
                            # the latency-critical kt/v/bias stream
                            nc.scalar.dma_start(
                                w_t[:],
                                wo[(2 * it + hh) * HD:(2 * it + hh + 1) * HD, :])
                            wo_tiles.append(w_t)
                    kt0 = kpool.tile([128, KCH], bf16, tag="kt")
                    nc.sync.dma_start(kt0[:], kT[b0][:, c * KCH:(c + 1) * KCH])
                    kt1 = kpool.tile([128, KCH], bf16, tag="kt")
                    nc.sync.dma_start(kt1[:], kT[b1][:, c * KCH:(c + 1) * KCH])
                    v0 = vpool.tile([128, 16 * VW], bf16, tag="vt")
                    nc.sync.dma_start(
                        v0[:].rearrange("p (n e) -> p n e", n=16),
                        vv[b0][:, c * 16:(c + 1) * 16, :])
                    v1 = vpool.tile([128, 16 * VW], bf16, tag="vt")
                    nc.sync.dma_start(
                        v1[:].rearrange("p (n e) -> p n e", n=16),
                        vv[b1][:, c * 16:(c + 1) * 16, :])
                    bias_sb = bpool.tile([128, KCH], bf16, tag="bias")
                    nc.sync.dma_start(bias_sb[:], bias[j][:, c, :])
                    if c == NCH - 1:
                        nc.vector.tensor_copy(kt0[:, KCH - 4:KCH],
                                              knT_sb[:, b0 * 4:b0 * 4 + 4])
                        nc.vector.tensor_copy(kt1[:, KCH - 4:KCH],
                                              knT_sb[:, b1 * 4:b1 * 4 + 4])
                        nc.gpsimd.dma_start(
                            v0[124:128, 15 * VW:15 * VW + HD],
                            vn_sb[b0 * 4:b0 * 4 + 4, :])
                        nc.gpsimd.dma_start(
                            v1[124:128, 15 * VW:15 * VW + HD],
                            vn_sb[b1 * 4:b1 * 4 + 4, :])
                    for n in range(4):
                        p_ps = pps.tile([128, 512], f32, tag="p")
                        for t in range(4):
                            ko = (n * 4 + t) * 128
                            nc.tensor.matmul(
                                p_ps[:, t * 128:t * 128 + ROWS],
                                kt0[:, ko:ko + 128],
                                qT_sb[:, b0 * ROWS:(b0 + 1) * ROWS],
                                start=True, stop=True)
                            nc.tensor.matmul(
                                p_ps[:, t * 128 + ROWS:(t + 1) * 128],
                                kt1[:, ko:ko + 128],
                                qT_sb[:, b1 * ROWS:(b1 + 1) * ROWS],
                                start=True, stop=True)
                        e_sb = apool.tile([128, 512], f32, tag="e")
                        nc.vector.tensor_tensor(
                            e_sb[:], p_ps[:], bias_sb[:, n * 512:(n + 1) * 512],
                            op=mybir.AluOpType.add)
                        a_bf = apool.tile([128, 512], bf16, tag="abf")
                        nc.scalar.activation(a_bf[:], e_sb[:], EXP)
                        for t in range(4):
                            kvt = c * 16 + n * 4 + t
                            first, last = (kvt == 0), (kvt == 63)
                            vo = (n * 4 + t) * VW
                            nc.tensor.matmul(
                                o_ps[0:ROWS, :],
                                a_bf[:, t * 128:t * 128 + ROWS],
                                v0[:, vo:vo + VW], start=first, stop=last)
                            nc.tensor.matmul(
                                o_ps[ROWS:128, :],
                                a_bf[:, t * 128 + ROWS:(t + 1) * 128],
                                v1[:, vo:vo + VW], start=first, stop=last,
                                tile_position=(0, 64))
                        if j == NPAIR - 1 and c == NCH - 1:
                            # HAM keep-warm: the last chunk's drain is
                            # DVE/ACT-paced with PE nearly idle, which lets
                            # the clock gate re-throttle to 1.2 GHz and the
                            # whole output projection then runs cold. These
                            # dummy matmuls (result unused) keep the PE
                            # activity window busy through the drain.
                            for _ in range(2):
                                d_ps = pps.tile([128, 512], f32, tag="p")
                                nc.tensor.matmul(d_ps[:, :], ident_b[:],
                                                 bias_sb[:, 0:512],
                                                 start=True, stop=True)
                _finalize_pair(tc, nc, mybir, apool, tps, j, o_ps, oT_sb,
                               ident_f)
                if j == NPAIR - 1:
                    for _ in range(3):
                        d_ps = pps.tile([128, 512], f32, tag="p")
                        nc.tensor.matmul(d_ps[:, :], ident_b[:],
                                         bias_sb[:, 0:512],
                                         start=True, stop=True)

        # ---------------- Phase O: output projection ------------------------
        with tc.tile_pool(name="outps", bufs=1, space="PSUM") as outps:
            out_ps = outps.tile([BQ, DIM], f32, tag="out")
            for h in range(H):
                w_t = wo_tiles[h]
                lhs = oT_sb[:, h * BQ:(h + 1) * BQ]
                for n in range(4):
                    nc.tensor.matmul(out_ps[:, n * 512:(n + 1) * 512], lhs,
                                     w_t[:, n * 512:(n + 1) * 512],
                                     start=(h == 0), stop=False)
            ones_r = ones16[0:1, :]
            for n in range(4):
                nc.tensor.matmul(out_ps[:, n * 512:(n + 1) * 512], ones_r,
                                 bo_sb[0:1, n * 512:(n + 1) * 512],
                                 start=False, stop=True)
            out_sb = const.tile([BQ, DIM], f32, tag="osb")
            nc.vector.tensor_copy(out_sb[:], out_ps[:])
            nc.sync.dma_start(out, out_sb[:])


def _finalize_pair(tc, nc, mybir, apool, tps, j, o_ps, oT_sb, ident_f):
    f32 = mybir.dt.float32
    recip = apool.tile([128, 1], f32, tag="recip")
    nc.vector.reciprocal(recip[:], o_ps[:, HD:HD + 1])
    o_sb = apool.tile([128, HD], f32, tag="osb")
    nc.vector.tensor_scalar_mul(o_sb[:], o_ps[:, 0:HD], recip[:])
    tr = tps.tile([128, 128], f32, tag="tr")
    nc.tensor.transpose(tr[:], o_sb[:], ident_f[:])
    oT_4d = oT_sb[:].rearrange("p (h b q) -> p h b q", h=H, b=BPC)
    for b2 in range(2):
        nc.vector.tensor_copy(
            oT_4d[:, :, 2 * j + b2, :],
            tr[:, b2 * ROWS:(b2 + 1) * ROWS].rearrange(
                "p (h q) -> p h q", h=H))


def _get_nc():
    if "nc" not in _CACHE:
        _CACHE["nc"] = _build()
    return _CACHE["nc"]


def kernel(x, attn_bias, cache_k, cache_v, wq, bq, wk, bk, wv, bv, wo, bo):
    import ml_dtypes
    from concourse.bass_utils import run_bass_kernel_spmd

    nc = _get_nc()
    scale = np.float32(1.0 / np.sqrt(HD))
    bf = ml_dtypes.bfloat16

    x = np.asarray(x, np.float32)
    xT_full = np.ascontiguousarray(x.reshape(B * Q, DIM).T).astype(bf)
    wq2 = np.ascontiguousarray(
        (np.asarray(wq, np.float32) * scale).reshape(DIM, H * HD)).astype(bf)
    bq2 = np.ascontiguousarray(
        (np.asarray(bq, np.float32) * scale).reshape(1, H * HD)).astype(bf)
    wk2 = np.asarray(wk, np.float32).astype(bf)
    bk2 = np.asarray(bk, np.float32).reshape(1, HD).astype(bf)
    wv2 = np.asarray(wv, np.float32).astype(bf)
    bv2 = np.asarray(bv, np.float32).reshape(1, HD).astype(bf)
    kTh = np.ascontiguousarray(
        np.roll(np.asarray(cache_k, np.float32), -Q, axis=1)
        .transpose(0, 2, 1)).astype(bf)
    vr0 = np.roll(np.asarray(cache_v, np.float32), -Q, axis=1)
    # [B, KV, HD] -> [B, 128, KV/128, HD+1]: per-partition-contiguous runs,
    # last column = 1.0 so the o-matmul accumulates softmax denominators
    vrh4 = vr0.reshape(B, KV // 128, 128, HD).transpose(0, 2, 1, 3)
    vrh = np.ones((B, 128, KV // 128, HD + 1), np.float32)
    vrh[..., :HD] = vrh4
    vrh = np.ascontiguousarray(vrh).astype(bf)
    # bias -> [pair, p, c, (n t r)] with kv = c*2048 + n*512 + t*128 + p
    ab = np.asarray(attn_bias, np.float32).reshape(B // 2, 2, ROWS, KV)
    abP = ab.transpose(0, 3, 1, 2).reshape(B // 2, KV, 2 * ROWS)
    biasP = np.ascontiguousarray(
        abP.reshape(B // 2, NCH, 4, 4, 128, 2 * ROWS)
        .transpose(0, 4, 1, 2, 3, 5)
        .reshape(B // 2, 128, NCH, KCH)).astype(bf)
    wo2 = np.asarray(wo, np.float32).reshape(H * HD, DIM).astype(bf)
    bo2 = np.asarray(bo, np.float32).reshape(1, DIM).astype(bf)

    in_maps = []
    for c in range(NCORES):
        in_maps.append({
            "xT": np.ascontiguousarray(xT_full[:, c * BQ:(c + 1) * BQ]),
            "wq": wq2, "bq": bq2, "wk": wk2, "bk": bk2, "wv": wv2, "bv": bv2,
            "kT": np.ascontiguousarray(kTh[c * BPC:(c + 1) * BPC]),
            "vv": np.ascontiguousarray(vrh[c * BPC:(c + 1) * BPC]),
            "bias": np.ascontiguousarray(biasP[NPAIR * c:NPAIR * (c + 1)]),
            "wo": wo2, "bo": bo2,
            "ones": np.ones((1, BQ), bf),
        })

    res = run_bass_kernel_spmd(nc, in_maps, core_ids=list(range(NCORES)))
    _CACHE["last_result"] = res
    outs = [res.results[c]["out"] for c in range(NCORES)]
    return np.concatenate(outs, axis=0).reshape(B, Q, DIM).astype(np.float32)
